# revision 38
# baseline (speedup 1.0000x reference)
"""Gomoku-style board feature kernel (18 channels, 32768 boards, 8 cores).

Bitboard design: each board row packs into a u32 word with both players
(my: bits 0-7, op: bits 16-23, guard bits between). All connectivity and
line features are computed as bitwise ops on [P=128, NB=32, 8] u32 tiles.

Output path: channels are expanded from the packed planes with fused
tensor_scalar (byte >> j) & 1 ops into a u16 channel arena (unit-stride
u16 src/dst hits the DVE 4x perf mode), written channel-major to HBM
(12 expansion channels as u16, 6 trivial channels as u8), and decoded
to the [B, 18, 8, 8] float32 result on the host (transpose + cast only).

Other key points:
- doubles channels: count>=2 <=> any bit set (marks come in windows of
  >=2 cells), so they reduce to an OR-fold + sign.
- scalar engine computes my/op planes via relu(+/-state) and the
  broadcast/cast channels; gpsimd is unused (no bitwise/compare support).
- line features reuse the conn direction chains (d2/d3 of row/col dirs).
"""
import numpy as np

import concourse.bass as bass
import concourse.bacc as bacc
import concourse.mybir as mybir
import concourse.tile as tile

Alu = mybir.AluOpType
Act = mybir.ActivationFunctionType
DT = mybir.dt

P = 128
NB = 32
NCORES = 8
BPC = P * NB
PAD = 18
R0 = 5
ROWS = slice(R0, R0 + 8)
SEG = 0x00FF00FF

# arena slot -> output channel: [2,3,4, 8,9,10, 11,12, 5,6,7, 13,14,15]
NCH_ARENA = 14


def _stt_raw(eng, out, in0, imm, in1, op0, op1, imm_dt=DT.uint32):
    outs = [eng.lower_ap(out)]
    return eng.add_instruction(
        mybir.InstTensorScalarPtr(
            name=eng.bass.get_next_instruction_name(),
            is_scalar_tensor_tensor=True,
            op0=op0, op1=op1,
            ins=[eng.lower_ap(in0),
                 mybir.ImmediateValue(dtype=imm_dt, value=imm),
                 eng.lower_ap(in1)],
            outs=outs,
        )
    )


def _stt(eng, out, in0, sh, op1, in1):
    if sh > 0:
        _stt_raw(eng, out, in0, sh, in1, Alu.logical_shift_left, op1)
    elif sh < 0:
        _stt_raw(eng, out, in0, -sh, in1, Alu.logical_shift_right, op1)
    else:
        eng.tensor_tensor(out, in0, in1, op1)


def feature_kernel(tc, out_d, state_d, side_d, allones):
    nc = tc.nc
    V, G, A = nc.vector, nc.gpsimd, nc.scalar

    state_v = state_d.rearrange("(p n) c -> p n c", p=P)
    # out_d: flat u8 [12*BPC*128 + 6*BPC*64]; first region = 12 expansion
    # channels as u16 (order: c1m c2m c3m l2m l3m r3m c1o c2o c3o l2o l3o r3o
    # -> device chans [2,3,4, 8,9,10, 5,6,7, 13,14,15]), second = 6 u8
    # channels [0, 1, 11, 12, 16, 17].
    o16 = out_d[0:12 * BPC * 128].bitcast(DT.uint16)
    out16_v = o16.rearrange("(c p x) -> p c x", c=12, p=P)
    o8 = out_d[12 * BPC * 128:12 * BPC * 128 + 6 * BPC * 64]
    out8_v = o8.rearrange("(c p x) -> p c x", c=6, p=P)

    pool_cm = tc.tile_pool(name="main", bufs=1)
    ipool_cm = tc.tile_pool(name="inp", bufs=1)
    with pool_cm as pool:
        ipool = ipool_cm.__enter__()
        # ---------- input ----------
        s = ipool.tile([P, NB, 64], DT.float32, name="s")
        nc.sync.dma_start(s[:, 0:NB // 2], state_v[:, 0:NB // 2])
        nc.sync.dma_start(s[:, NB // 2:NB], state_v[:, NB // 2:NB])
        myf = ipool.tile([P, NB, 64], DT.float32, name="myf")
        opf = ipool.tile([P, NB, 64], DT.float32, name="opf")
        H = NB // 2
        if allones:
            sgn = s[:]
        else:
            side_v = side_d.rearrange("(p n) -> p n", p=P)
            sideT = ipool.tile([P, NB], DT.float32, name="sideT")
            nc.sync.dma_start(sideT[:], side_v)
            sgn = ipool.tile([P, NB, 64], DT.float32, name="sp")
            G.tensor_tensor(
                sgn[:], s[:], sideT[:, :, None].broadcast_to((P, NB, 64)), Alu.mult
            )
            sgn = sgn[:]
        # half-split relus so vector packing can start sooner
        A.activation(myf[:, 0:H], sgn[:, 0:H], Act.Relu)
        A.activation(myf[:, H:NB], sgn[:, H:NB], Act.Relu)
        A.activation(opf[:, 0:H], sgn[:, 0:H], Act.Relu, scale=-1.0)
        A.activation(opf[:, H:NB], sgn[:, H:NB], Act.Relu, scale=-1.0)

        # ch0/ch1 u16 direct relus (r-major cell order; host skips the j/r
        # swap for these two channels)
        chA = pool.tile([P, 2, NB, 64], DT.uint8, name="chA")
        A.activation(chA[:, 0], sgn, Act.Relu)
        A.activation(chA[:, 1], sgn, Act.Relu, scale=-1.0)
        nc.sync.dma_start(out8_v[:, 0:2, :],
                          chA.rearrange("p c n x -> p c (n x)"))

        # ---------- guard-zero memsets (vector, overlaps input latency) ----
        PLN = pool.tile([P, 4, NB, PAD], DT.uint32, name="PLN")  # Ad Bd Ed Nd
        V.memset(PLN[:, :, :, 4:5], 0)
        V.memset(PLN[:, :, :, 13:18], 0)
        # padded conn temps: d2c, d3c (dir 1 keepalive), dp2, dp3, dp4, tp3, tp4
        CT = pool.tile([P, 7, NB, PAD], DT.uint32, name="CT")
        V.memset(CT[:, :, :, 4:5], 0)
        V.memset(CT[:, :, :, 13:16], 0)
        # col line-feature arena, 12 rows each, valid rows 2..10.
        # guard needs: u read at +2 (rows 10,11); d read at -2 (rows 0,1);
        # b,q,c,i1,e,g1,g2,l3 read at -1 (row 1 only)
        CNAMES = ["u", "b", "c", "d", "e", "y", "i1", "j1", "g1", "l3",
                  "g2", "q", "a", "w", "r1", "lb", "d0", "d1", "md", "o3",
                  "rb", "x", "l2", "r3"]
        CIDX = {n: i for i, n in enumerate(CNAMES)}
        LFC = pool.tile([P, len(CNAMES), NB, 12], DT.uint32, name="LFC")
        V.memset(LFC[:, 0:1, :, 10:12], 0)
        V.memset(LFC[:, 3:4, :, 0:1], 0)
        V.memset(LFC[:, 1:12, :, 1:2], 0)
        lbmC = pool.tile([P, 8], DT.uint32, name="lbmC")
        rbmC = pool.tile([P, 8], DT.uint32, name="rbmC")
        V.memset(lbmC[:], 0)
        V.memset(lbmC[:, 0:1], SEG)
        V.memset(rbmC[:], 0)
        V.memset(rbmC[:, 3:8], SEG)

        # ---------- packing ----------
        myR = pool.tile([P, NB, 8], DT.uint32, name="myR")
        opR = pool.tile([P, NB, 8], DT.uint32, name="opR")
        pk1 = ipool.tile([P, NB * 8, 4], DT.float32, name="pk1")
        pk2 = ipool.tile([P, NB * 8, 2], DT.float32, name="pk2")

        def pack(dst, srcf, n0, n1):
            v = srcf[:, n0:n1].rearrange("p n (r j2 t) -> p (n r) j2 t",
                                         t=2, j2=4)
            a1, b1 = v[:, :, :, 1], v[:, :, :, 0]
            t1 = pk1[:, n0 * 8:n1 * 8]
            V.scalar_tensor_tensor(t1, a1, 2.0, b1, op0=Alu.mult, op1=Alu.add)
            w2 = t1.rearrange("p q (k t) -> p q k t", t=2)
            a2, b2 = w2[:, :, :, 1], w2[:, :, :, 0]
            t2 = pk2[:, n0 * 8:n1 * 8]
            V.scalar_tensor_tensor(t2, a2, 4.0, b2, op0=Alu.mult, op1=Alu.add)
            w3 = t2.rearrange("p (n r) t -> p n r t", r=8)
            a3, b3 = w3[:, :, :, 1], w3[:, :, :, 0]
            V.scalar_tensor_tensor(dst[:, n0:n1], a3, 16.0, b3,
                                   op0=Alu.mult, op1=Alu.add)

        pack(myR, myf, 0, H)
        pack(myR, myf, H, NB)
        pack(opR, opf, 0, H)
        pack(opR, opf, H, NB)
        ipool_cm.__exit__(None, None, None)
        lpool_cm = tc.tile_pool(name="late", bufs=1)
        lpool = lpool_cm.__enter__()

        # ---------- planes ----------
        Ad, Bd, Ed, Nd = PLN[:, 0], PLN[:, 1], PLN[:, 2], PLN[:, 3]
        _stt(V, Ad[:, :, ROWS], opR[:], 16, Alu.bitwise_or, myR[:])
        _stt(V, Bd[:, :, ROWS], myR[:], 16, Alu.bitwise_or, opR[:])
        V.tensor_tensor(Ed[:, :, ROWS], Ad[:, :, ROWS], Bd[:, :, ROWS],
                        Alu.bitwise_or)
        V.tensor_scalar(Ed[:, :, ROWS], Ed[:, :, ROWS], SEG, None, Alu.bitwise_xor)
        V.tensor_scalar(Nd[:, :, ROWS], Ad[:, :, ROWS], SEG, None, Alu.bitwise_xor)

        # ---------- connectivity ----------
        # A2/A3/A4: dir-major arenas
        A2 = pool.tile([P, 4, NB, 8], DT.uint32, name="A2")
        A3 = pool.tile([P, 4, NB, 8], DT.uint32, name="A3")
        A4 = pool.tile([P, 4, NB, 8], DT.uint32, name="A4")
        # unpadded temps for dir 0 (bit shifts only)
        d2r = pool.tile([P, NB, 8], DT.uint32, name="d2r")
        d3r = pool.tile([P, NB, 8], DT.uint32, name="d3r")
        u4 = pool.tile([P, NB, 8], DT.uint32, name="u4")
        u5 = pool.tile([P, NB, 8], DT.uint32, name="u5")
        u6 = pool.tile([P, NB, 8], DT.uint32, name="u6")
        mv = Ad[:, :, ROWS]

        # dir 0: (0,1) -- keep d2r/d3r for row line features
        _stt(V, d2r[:], mv, 1, Alu.bitwise_and, mv)
        _stt(V, d3r[:], d2r[:], 1, Alu.bitwise_and, d2r[:])
        _stt(V, u4[:], d3r[:], 1, Alu.bitwise_and, d3r[:])      # d4
        _stt(V, A2[:, 0], d2r[:], -1, Alu.bitwise_or, d2r[:])
        _stt(V, u5[:], d3r[:], -1, Alu.bitwise_or, d3r[:])      # t3
        _stt(V, A3[:, 0], d3r[:], -2, Alu.bitwise_or, u5[:])
        _stt(V, u6[:], u4[:], -1, Alu.bitwise_or, u4[:])        # t4
        _stt(V, A4[:, 0], u6[:], -2, Alu.bitwise_or, u6[:])

        # dir 1: (1,0) -- keep d2c/d3c (padded) for col line features
        d2c, d3c = CT[:, 0], CT[:, 1]
        dp2, dp3, dp4, tp3, tp4 = CT[:, 2], CT[:, 3], CT[:, 4], CT[:, 5], CT[:, 6]

        def fwd(t, di):
            return t[:, :, R0 - di:R0 + 8 - di]

        def bwd(t, di, k=1):
            return t[:, :, R0 + k * di:R0 + 8 + k * di]

        def conn_dir(di_i, di, dj, td2, td3):
            _stt(V, td2[:, :, ROWS], fwd(Ad, di), dj, Alu.bitwise_and, mv)
            _stt(V, td3[:, :, ROWS], fwd(td2, di), dj, Alu.bitwise_and,
                 td2[:, :, ROWS])
            _stt(V, dp4[:, :, ROWS], fwd(td3, di), dj, Alu.bitwise_and,
                 td3[:, :, ROWS])
            _stt(V, A2[:, di_i], bwd(td2, di), -dj, Alu.bitwise_or,
                 td2[:, :, ROWS])
            _stt(V, tp3[:, :, ROWS], bwd(td3, di), -dj, Alu.bitwise_or,
                 td3[:, :, ROWS])
            _stt(V, A3[:, di_i], bwd(td3, di, 2), -2 * dj, Alu.bitwise_or,
                 tp3[:, :, ROWS])
            _stt(V, tp4[:, :, ROWS], bwd(dp4, di), -dj, Alu.bitwise_or,
                 dp4[:, :, ROWS])
            _stt(V, A4[:, di_i], bwd(tp4, di, 2), -2 * dj, Alu.bitwise_or,
                 tp4[:, :, ROWS])

        conn_dir(1, 1, 0, d2c, d3c)  # dir (1,0), keeps d2c/d3c for col mode
        Rg = pool.tile([P, 6, NB, 8], DT.uint32, name="Rg")

        # ---------- channel arena ----------
        # arena cell order is j-major: [ch, board, j, r]; host swaps j/r.
        # slot -> channel: [2,3,4, 8,9,10, 11,12, 5,6,7, 13,14,15]
        arena = lpool.tile([P, 12, NB, 64], DT.uint16, name="arena")
        rgb = Rg.bitcast(DT.uint8).rearrange("p c n (r b) -> p c n r b", b=4)
        # unit-stride u16 row-byte planes: [persp, plane, board, row-word]
        RB = lpool.tile([P, 2, 6, NB, 8], DT.uint16, name="RB")

        def compact(persp, pl0, npl):
            A.activation(RB[:, persp, pl0:pl0 + npl],
                         rgb[:, pl0:pl0 + npl, :, :, 2 * persp], Act.Copy)

        def expand(slot0, persp, pl0, npl):
            """arena[:, slot0+c, :, j*8+r] = (RB[persp, pl0+c] >> j) & 1"""
            av = arena.rearrange("p c n (j r) -> p c n j r", r=8)
            for j in range(8):
                V.tensor_scalar(
                    av[:, slot0:slot0 + npl, :, j, :],
                    RB[:, persp, pl0:pl0 + npl],
                    j, 1, op0=Alu.logical_shift_right, op1=Alu.bitwise_and)

        # ---------- line features ----------
        # row mode: bit-shift ops on [P, NB, 8] u32, reusing d2r/d3r
        me, op_, em, nm = mv, Bd[:, :, ROWS], Ed[:, :, ROWS], Nd[:, :, ROWS]
        R = {}

        def rt(n):
            if n not in R:
                R[n] = pool.tile([P, NB, 8], DT.uint32, name="r_" + n)
            return R[n][:]

        # l2 chain: u, w, a'=(u>>1)&d2, b=(d2>>2)&w, y=(b<<1)|b, a=a'>>1,
        #           q=a|y, l2=(q<<1)|a
        _stt(V, rt("u"), em, -1, Alu.bitwise_and, em)
        _stt(V, rt("w"), em, -3, Alu.bitwise_and, em)
        _stt(V, rt("ap"), rt("u"), -1, Alu.bitwise_and, d2r[:])
        _stt(V, rt("b"), d2r[:], -2, Alu.bitwise_and, rt("w"))
        _stt(V, rt("y"), rt("b"), 1, Alu.bitwise_or, rt("b"))
        V.tensor_scalar(rt("a"), rt("ap"), 1, SEG,
                        op0=Alu.logical_shift_right, op1=Alu.bitwise_and)
        V.tensor_tensor(rt("q"), rt("a"), rt("y"), Alu.bitwise_or)
        _stt(V, rt("l2"), rt("q"), 1, Alu.bitwise_or, rt("a"))
        # l3/r3 chains with batched (x<<1)|x ops via RA arena:
        # slots 0=c 1=d 2=e 3=i1 4=j1 5=g1 6=l3 7=g2
        RA = pool.tile([P, 8, NB, 8], DT.uint32, name="RA")
        _stt(V, rt("r1"), em, -4, Alu.bitwise_and, em)
        _stt(V, RA[:, 0], d3r[:], -3, Alu.bitwise_and, rt("r1"))   # c
        V.tensor_scalar(rt("lb"), op_, 1, 0x00010001,
                        op0=Alu.logical_shift_left, op1=Alu.bitwise_or)
        _stt(V, rt("d0"), em, -1, Alu.bitwise_and, d3r[:])      # (em>>1)&d3
        _stt(V, rt("d1"), nm, -2, Alu.bitwise_and, rt("d0"))    # (nm>>2)&d0'
        _stt(V, RA[:, 1], rt("d1"), -2, Alu.bitwise_and, rt("lb"))  # d
        _stt(V, rt("o3"), d3r[:], -3, Alu.bitwise_and, nm)
        _stt(V, rt("o3"), nm, -4, Alu.bitwise_and, rt("o3"))
        V.tensor_scalar(rt("rb"), op_, 5, 0x00F800F8,
                        op0=Alu.logical_shift_right, op1=Alu.bitwise_or)
        V.tensor_tensor(rt("x"), rt("lb"), rt("rb"), Alu.bitwise_xor)
        V.tensor_tensor(RA[:, 2], rt("o3"), rt("x"), Alu.bitwise_and)  # e
        _stt(V, RA[:, 3:6], RA[:, 0:3], 1, Alu.bitwise_or, RA[:, 0:3])
        _stt(V, RA[:, 6:8], RA[:, 3:6:2], 1, Alu.bitwise_or, RA[:, 0:3:2])
        _stt(V, rt("md"), RA[:, 1], 2, Alu.bitwise_or, RA[:, 4])
        _stt(V, rt("r3"), RA[:, 7], 1, Alu.bitwise_or, rt("md"))

        # col mode: row-offset TT ops; t = dn(d2c,1), m3 = dn(d3c,2)
        R0T = 2

        def T(n, k=0):
            t = LFC[:, CIDX[n]]
            return t[:, :, R0T + k:R0T + 8 + k]

        def dn(x, k):
            return x[:, :, R0 + k:R0 + 8 + k]

        def MV(x, k=0):
            return x[:, :, R0 + k:R0 + 8 + k]

        tcol = dn(d2c, 1)     # me[r]&me[r+1]
        m3col_p1 = dn(d3c, 3)  # m3 at +1 row = d3c[r+3]
        V.tensor_tensor(T("u"), MV(Ed), dn(Ed, 1), Alu.bitwise_and)
        V.tensor_tensor(T("a"), tcol, T("u", 2), Alu.bitwise_and)
        V.tensor_tensor(T("w"), MV(Ed), dn(Ed, 3), Alu.bitwise_and)
        V.tensor_tensor(T("b"), T("w"), dn(d2c, 2), Alu.bitwise_and)  # w & t(+1)
        V.tensor_tensor(T("r1"), MV(Ed), dn(Ed, 4), Alu.bitwise_and)
        V.tensor_tensor(T("c"), T("r1"), m3col_p1, Alu.bitwise_and)
        V.tensor_tensor(
            T("lb"), MV(Bd, -1),
            lbmC[:, None, :].broadcast_to((P, NB, 8)), Alu.bitwise_or)
        V.tensor_tensor(T("d0"), dn(d3c, 2), dn(Ed, 3), Alu.bitwise_and)
        V.tensor_tensor(T("d1"), T("d0"), dn(Nd, 4), Alu.bitwise_and)
        V.tensor_tensor(T("d"), T("d1"), T("lb"), Alu.bitwise_and)
        V.tensor_tensor(T("o3"), m3col_p1, MV(Nd), Alu.bitwise_and)
        V.tensor_tensor(T("o3"), T("o3"), dn(Nd, 4), Alu.bitwise_and)
        V.tensor_tensor(
            T("rb"), MV(Bd, 5),
            rbmC[:, None, :].broadcast_to((P, NB, 8)), Alu.bitwise_or)
        V.tensor_tensor(T("x"), T("lb"), T("rb"), Alu.bitwise_xor)
        V.tensor_tensor(T("e"), T("o3"), T("x"), Alu.bitwise_and)
        # batched: [y,i1,j1,g1] = [b,c,d,e] | [b,c,d,e](-1 row)
        V.tensor_tensor(LFC[:, 5:9, :, 2:10], LFC[:, 1:5, :, 2:10],
                        LFC[:, 1:5, :, 1:9], Alu.bitwise_or)
        V.tensor_tensor(T("q"), T("a"), T("y"), Alu.bitwise_or)
        V.tensor_tensor(T("l2"), T("a"), T("q", -1), Alu.bitwise_or)
        # batched: [l3,g2] = [c,e] | [i1,g1](-1 row)
        V.tensor_tensor(LFC[:, 9:11, :, 2:10], LFC[:, 2:5:2, :, 2:10],
                        LFC[:, 6:9:2, :, 1:9], Alu.bitwise_or)
        V.tensor_tensor(T("md"), T("j1"), T("d", -2), Alu.bitwise_or)
        V.tensor_tensor(T("r3"), T("md"), T("g2", -1), Alu.bitwise_or)

        # merges into Rg[3..5]
        V.tensor_tensor(Rg[:, 3], rt("l2"), T("l2"), Alu.bitwise_or)
        _stt(V, Rg[:, 4], RA[:, 6], 1, Alu.bitwise_or, T("l3", -1))
        V.tensor_tensor(Rg[:, 5], rt("r3"), T("r3"), Alu.bitwise_or)

        # ---------- doubles ----------
        orf2 = pool.tile([P, NB], DT.uint32, name="orf2")
        orf3 = pool.tile([P, NB], DT.uint32, name="orf3")
        lr = pool.tile([P, NB, 8], DT.uint32, name="lr")
        V.tensor_reduce(orf2[:], Rg[:, 3], axis=mybir.AxisListType.X,
                        op=Alu.bitwise_or)
        V.tensor_tensor(lr[:], Rg[:, 4], Rg[:, 5], Alu.bitwise_or)
        V.tensor_reduce(orf3[:], lr[:], axis=mybir.AxisListType.X,
                        op=Alu.bitwise_or)
        dgef = pool.tile([P, 4, NB], DT.float32, name="dgef")
        o2b = orf2.bitcast(DT.uint8)
        o3b = orf3.bitcast(DT.uint8)
        A.activation(dgef[:, 0], o2b[:, 0::4], Act.Sign)  # ch11
        A.activation(dgef[:, 1], o3b[:, 0::4], Act.Sign)  # ch12
        A.activation(dgef[:, 2], o2b[:, 2::4], Act.Sign)  # ch16
        A.activation(dgef[:, 3], o3b[:, 2::4], Act.Sign)  # ch17

        # ---------- line expansion (early: only needs conn dirs 0,1) -----
        compact(0, 3, 3)
        expand(3, 0, 3, 3)   # line my -> u16 chans 3:6
        nc.sync.dma_start(out16_v[:, 3:6, :],
                          arena[:, 3:6].rearrange("p c n x -> p c (n x)"))
        compact(1, 3, 3)
        expand(9, 1, 3, 3)   # line op -> u16 chans 9:12
        nc.sync.dma_start(out16_v[:, 9:12, :],
                          arena[:, 9:12].rearrange("p c n x -> p c (n x)"))
        # doubles broadcasts (scalar engine) -> u8 chans 2:6
        chB = lpool.tile([P, 4, NB, 64], DT.uint8, name="chB")
        for k in range(4):
            A.activation(chB[:, k],
                         dgef[:, k, :, None].broadcast_to((P, NB, 64)),
                         Act.Copy)
        nc.sync.dma_start(out8_v[:, 2:6, :],
                          chB.rearrange("p c n x -> p c (n x)"))

        # ---------- conn dirs 2,3 + merges + conn expansion ----------
        conn_dir(2, 1, 1, dp2, dp3)
        conn_dir(3, 1, -1, dp2, dp3)
        x2 = pool.tile([P, 4, NB, 8], DT.uint32, name="x2")
        f2 = pool.tile([P, 2, NB, 8], DT.uint32, name="f2")
        # c1 = mv ^ AND(all a2)
        V.tensor_tensor(f2[:], A2[:, 0:2], A2[:, 2:4], Alu.bitwise_and)
        V.tensor_tensor(x2[:, 0], f2[:, 0], f2[:, 1], Alu.bitwise_and)
        V.tensor_tensor(Rg[:, 0], mv, x2[:, 0], Alu.bitwise_xor)
        # c2 = OR(a2^a3)
        V.tensor_tensor(x2[:], A2[:], A3[:], Alu.bitwise_xor)
        V.tensor_tensor(f2[:], x2[:, 0:2], x2[:, 2:4], Alu.bitwise_or)
        V.tensor_tensor(Rg[:, 1], f2[:, 0], f2[:, 1], Alu.bitwise_or)
        # c3 = OR(a3^a4)
        V.tensor_tensor(x2[:], A3[:], A4[:], Alu.bitwise_xor)
        V.tensor_tensor(f2[:], x2[:, 0:2], x2[:, 2:4], Alu.bitwise_or)
        V.tensor_tensor(Rg[:, 2], f2[:, 0], f2[:, 1], Alu.bitwise_or)
        # conn compacts inline on vector (A would add latency here)
        V.tensor_scalar(RB[:, 0, 0:3], rgb[:, 0:3, :, :, 0], 0, None, Alu.add)
        V.tensor_scalar(RB[:, 1, 0:3], rgb[:, 0:3, :, :, 2], 0, None, Alu.add)
        expand(0, 0, 0, 3)   # conn my -> u16 chans 0:3
        nc.sync.dma_start(out16_v[:, 0:3, :],
                          arena[:, 0:3].rearrange("p c n x -> p c (n x)"))
        expand(6, 1, 0, 3)   # conn op -> u16 chans 6:9
        nc.sync.dma_start(out16_v[:, 6:9, :],
                          arena[:, 6:9].rearrange("p c n x -> p c (n x)"))
        lpool_cm.__exit__(None, None, None)


_NC_CACHE = {}


def _build_nc(allones):
    if allones in _NC_CACHE:
        return _NC_CACHE[allones]
    nc = bacc.Bacc("TRN2", debug=False, enable_asserts=False)
    state_d = nc.dram_tensor("state", [BPC, 64], DT.float32, kind="ExternalInput").ap()
    side_d = nc.dram_tensor("side", [BPC], DT.float32, kind="ExternalInput").ap()
    out_d = nc.dram_tensor("out", [12 * BPC * 128 + 6 * BPC * 64], DT.uint8, kind="ExternalOutput").ap()
    with tile.TileContext(nc) as tc:
        feature_kernel(tc, out_d, state_d, side_d, allones)
    nc.finalize()
    _NC_CACHE[allones] = nc
    return nc


_JIT_CACHE = {}


def _get_runner(allones):
    if allones in _JIT_CACHE:
        return _JIT_CACHE[allones]
    import jax
    from jax.sharding import Mesh, PartitionSpec, NamedSharding
    try:
        from jax.experimental.shard_map import shard_map
    except ImportError:
        from jax.shard_map import shard_map  # newer jax
    from concourse import bass2jax as B2J

    B2J.install_neuronx_cc_hook()
    nc = _build_nc(allones)

    in_names = ["state", "side"]
    out_names = ["out"]
    out_avals = [jax.core.ShapedArray((12 * BPC * 128 + 6 * BPC * 64,), np.uint8)]
    all_names = in_names + out_names
    if nc.partition_id_tensor is not None:
        all_names = all_names + [nc.partition_id_tensor.name]

    def _body(state_a, side_a, zeros_a):
        operands = [state_a, side_a, zeros_a]
        if nc.partition_id_tensor is not None:
            operands.append(B2J.partition_id_tensor())
        outs = B2J._bass_exec_p.bind(
            *operands,
            out_avals=tuple(out_avals),
            in_names=tuple(all_names),
            out_names=tuple(out_names),
            lowering_input_output_aliases=(),
            sim_require_finite=True,
            sim_require_nnan=True,
            nc=nc,
        )
        return outs[0]

    devices = jax.devices()[:NCORES]
    mesh = Mesh(np.asarray(devices), ("core",))
    spec = PartitionSpec("core")
    sharded = jax.jit(
        shard_map(
            _body, mesh=mesh,
            in_specs=(spec, spec, spec),
            out_specs=spec,
            check_rep=False,
        ),
        donate_argnums=(2,),
        keep_unused=True,
    )

    def put(shards):
        arrs = [jax.device_put(s, devices[i]) for i, s in enumerate(shards)]
        global_shape = (sum(s.shape[0] for s in shards),) + shards[0].shape[1:]
        return jax.make_array_from_single_device_arrays(
            global_shape, NamedSharding(mesh, spec), arrs
        )

    _JIT_CACHE[allones] = (sharded, put)
    return _JIT_CACHE[allones]


def kernel(state, side):
    """Full-input entry point: state [32768,8,8] f32, side [32768] f32."""
    state = np.ascontiguousarray(np.asarray(state, dtype=np.float32)).reshape(-1, 64)
    side = np.ascontiguousarray(np.asarray(side, dtype=np.float32)).reshape(-1)
    B = state.shape[0]
    assert B == BPC * NCORES, (B, BPC * NCORES)
    allones = bool(np.all(side == 1.0))
    sharded, put = _get_runner(allones)
    state_g = put([state[i * BPC:(i + 1) * BPC] for i in range(NCORES)])
    side_g = put([side[i * BPC:(i + 1) * BPC] for i in range(NCORES)])
    zeros_g = put([np.zeros((12 * BPC * 128 + 6 * BPC * 64,), np.uint8) for _ in range(NCORES)])
    out = sharded(state_g, side_g, zeros_g)
    out = np.asarray(out).reshape(NCORES, -1)
    n16 = 12 * BPC * 128
    exp = (out[:, :n16].copy().view(np.uint16)
           .reshape(NCORES, 12, BPC, 8, 8))
    side_ch = out[:, n16:].reshape(NCORES, 6, BPC, 8, 8)
    res = np.empty((NCORES, BPC, 18, 8, 8), np.float32)
    # expansion channels are [j, r] cell order on device; swap to [r, j]
    expt = exp.transpose(0, 2, 1, 4, 3).astype(np.float32)
    res[:, :, [2, 3, 4, 8, 9, 10, 5, 6, 7, 13, 14, 15]] = expt
    # u8 side channels are r-major already
    res[:, :, [0, 1, 11, 12, 16, 17]] = (
        side_ch.transpose(0, 2, 1, 3, 4).astype(np.float32))
    return res.reshape(NCORES * BPC, 18, 8, 8)
